# revision 19
# baseline (speedup 1.0000x reference)
"""Causal attention (B=4, L=2048, d_model=1024, d_k=d_v=128) on 8 TRN2 NeuronCores.

Sharding (SPMD — one program, per-core data): core c -> batch b = c//2,
KEY parity kp = c%2.  Each core computes, for ALL 2048 queries of its
batch, the partial softmax numerator and denominator over ITS key parity
(key blocks j == kp mod 2); the host adds the two partials of a pair and
divides.  This avoids duplicating the K/V projections (the previous
query-parity scheme projected K/V twice per pair); only Q — a single
projection — is duplicated.  PE work/core: 52.3k cycles vs 61.5k.

Data layout: the host packs X^T [d_model, L] chunk-major ([p, c, l]) and,
within every 256-column pair of q-blocks, puts this core's KEY block
first ([key | other]; identity for even cores, swapped for odd).  The
causal structure is then core-uniform: key-slot k' (key block 2k'+kp)
covers column positions >= 2k', with a triangular mask at position 2k'
(the diagonal, both parities) and an all-or-nothing mask at 2k'+1 (live
on even cores, dead on odd) — fed as mask data.  The host un-permutes
the output positions.

Within a core (all matmuls contract on the partition dim):
  - Throwaway matmuls on a zeroed tile run first (and fill early DMA-wait
    bubbles) so the PE_HAM clock gate reaches 2.4 GHz before real work.
  - X streams as 8 pieces (256 cols, 0.5 MB) into slices of ONE SBUF
    tile; K/V projections consume the key columns through a stride-256
    view (free dim 256 over a piece pair), Q reads 512-wide groups.
  - Scores TRANSPOSED: S^T[key, q] = K^T.T @ Q^T; 1/sqrt(d_k) folded
    into W_Q host-side.  exp (Scalar) writes A^T to SBUF bf16; boundary
    masks are applied POST-exp as 0/1 multiplies on GpSimd (|s| <= ~12,
    exp cannot overflow).  PSUM->SBUF copies: K on Scalar, Q/V on Vector.
  - V is augmented with a ones column: Z_aug = A^T.T @ [V | 1] gives the
    partial denominator in column 128 for free.  No row-max subtraction.
  - Per round r (piece pair): Q group r, K/V pair r, V transposes,
    scores (k' <= 2r+1, group r), exps, AV positions 4r..4r+3, output
    DMA — so compute and output ride just behind the input stream.
"""

import os
import sys

sys.path.insert(0, "/opt/trn_rl_repo")
sys.path.insert(0, "/opt/trn_rl_repo/concourse")

import ml_dtypes
import numpy as np

import concourse.bass as bass  # noqa: F401
import concourse.mybir as mybir
import concourse.tile as tile
from concourse import bacc
from concourse.bass_utils import run_bass_kernel_spmd
from concourse.masks import make_identity

B, L, DM, DK, DV = 4, 2048, 1024, 128, 128
NB = L // 128    # 16 q-blocks (column positions) per batch
KS = 8           # key slots per core (one parity)
NCH = DM // 128  # 8 d_model chunks
SCALE = float(DK) ** -0.5

COMPUTE = os.environ.get("ATTN_COMPUTE", "bf16")  # "bf16" | "f32"
N_WARM = int(os.environ.get("ATTN_WARM", "8"))

F32 = mybir.dt.float32


def _cdt():
    return mybir.dt.bfloat16 if COMPUTE == "bf16" else mybir.dt.float32


def _np_cdt():
    return ml_dtypes.bfloat16 if COMPUTE == "bf16" else np.float32


def build_nc():
    cdt = _cdt()
    nc = bacc.Bacc()

    # weights pre-arranged on host: [p, c*128+d] = W[c*128+p, d]
    wq_ext = nc.declare_dram_parameter("wq", [128, DM], cdt, isOutput=False)
    wk_ext = nc.declare_dram_parameter("wk", [128, DM], cdt, isOutput=False)
    wv_ext = nc.declare_dram_parameter("wv", [128, DM], cdt, isOutput=False)
    # X pieces: piece i = column positions 256i..256i+255 (q-block pair),
    # packed [p, c*256+l] (contiguous DRAM run per partition)
    xp_ext = [
        nc.declare_dram_parameter(f"xp{i}", [128, NCH * 256], cdt,
                                  isOutput=False)
        for i in range(8)
    ]
    # multiplicative masks: col block 0 = triangle (diagonal, both
    # parities), col block 1 = all-ones (even cores) / all-zeros (odd)
    mask_ext = nc.declare_dram_parameter("maskT", [128, 256], cdt,
                                         isOutput=False)
    # partial [numerator | denominator] per column position
    out_ext = nc.declare_dram_parameter("out", [NB * 128, DV + 1], F32,
                                        isOutput=True)

    with tile.TileContext(nc) as tc:
        with (
            tc.tile_pool(name="persist", bufs=1) as persist,
            tc.tile_pool(name="mm_ps", bufs=6, space="PSUM") as mm_ps,
            tc.tile_pool(name="z_ps", bufs=2, space="PSUM") as z_ps,
            tc.tile_pool(name="work", bufs=6) as work,
        ):
            # ---- PE warm-up (z_ps is free until the AV phase) ----
            warm_t = persist.tile([128, 512], cdt, tag="warmt")
            nc.gpsimd.memset(warm_t[:], 0.0)

            def warm(n):
                for _ in range(n):
                    wp = z_ps.tile([128, 512], F32, tag="z", name="warm")
                    nc.tensor.matmul(wp[:], warm_t[:, 0:128], warm_t[:],
                                     start=True, stop=True)

            warm(N_WARM)

            ident = persist.tile([128, 128], cdt, tag="ident")
            make_identity(nc, ident)

            # ---- input DMAs: one sync ring, weights interleaved ----
            w_sb = {}

            def load_w(name, ext):
                t = persist.tile([128, NCH, 128], cdt, tag=name, name=name)
                nc.sync.dma_start(
                    out=t[:], in_=ext.rearrange("p (c d) -> p c d", d=128)
                )
                w_sb[name] = t

            # X lands in one tile per ROUND (q-group) so dependency
            # tracking stays piece-granular while K/V can still read a
            # stride-256 view across the round's two pieces
            xr = [persist.tile([128, NCH, 512], cdt, tag=f"xr{r}", name=f"xr{r}")
                  for r in range(4)]

            def load_piece(i):
                nc.sync.dma_start(
                    out=xr[i // 2][:, :, (i % 2) * 256:(i % 2) * 256 + 256],
                    in_=xp_ext[i].rearrange("p (c l) -> p c l", l=256),
                )

            load_w("wq", wq_ext)
            load_piece(0)
            load_w("wk", wk_ext)
            load_piece(1)
            load_w("wv", wv_ext)
            for i in range(2, 8):
                load_piece(i)
            mask_sb = persist.tile([128, 256], cdt, tag="mask")
            nc.gpsimd.dma_start(out=mask_sb[:], in_=mask_ext[:])

            # key-column view of a round tile: position pairs are
            # [key | other], so key blocks are the even 128-blocks
            xkey = [t.rearrange("p c (k l) -> p c k l", l=256) for t in xr]

            # ---- persistent per-stage tiles ----
            qt = [persist.tile([128, 512], cdt, tag=f"qt{g}", name=f"qt{g}")
                  for g in range(4)]
            kt = [persist.tile([128, 512], cdt, tag=f"kt{g}", name=f"kt{g}")
                  for g in range(2)]
            vt = [persist.tile([128, 512], cdt, tag=f"vt{g}", name=f"vt{g}")
                  for g in range(2)]
            v_aug = []
            for m in range(KS):
                t = persist.tile([128, DV + 1], cdt, tag=f"va{m}", name=f"va{m}")
                nc.vector.memset(t[:, DV:DV + 1], 1.0)
                v_aug.append(t)
            at = {}
            for k in range(KS):
                for g in range(k // 2, 4):
                    at[(k, g)] = persist.tile([128, 512], cdt, tag=f"at{k}_{g}",
                                              name=f"at{k}_{g}")

            vcopy = nc.vector.tensor_copy
            scopy = nc.scalar.copy

            def qproj(g):
                ps = mm_ps.tile([128, 512], F32, tag="mm", name=f"q{g}")
                for c in range(NCH):
                    nc.tensor.matmul(
                        ps[:], w_sb["wq"][:, c, :],
                        xr[g][:, c, :],
                        start=(c == 0), stop=(c == NCH - 1),
                    )
                vcopy(qt[g][:], ps[:])

            def kvproj(pr):
                # key slots 2pr, 2pr+1 from round-pr pieces (256 free)
                for name, dst, cp in (("wk", kt, scopy), ("wv", vt, vcopy)):
                    ps = mm_ps.tile([128, 256], F32, tag="mm",
                                    name=f"{name}{pr}")
                    for c in range(NCH):
                        nc.tensor.matmul(
                            ps[:], w_sb[name][:, c, :],
                            xkey[pr][:, c, :, 0:128],
                            start=(c == 0), stop=(c == NCH - 1),
                        )
                    cp(dst[pr // 2][:, (pr % 2) * 256:(pr % 2) * 256 + 256],
                       ps[:])

            def vt_blocks(ms):
                for m in ms:
                    vps = mm_ps.tile([128, 128], cdt, tag="mm", name="vps")
                    nc.tensor.transpose(
                        vps[:],
                        vt[m // 4][:, (m % 4) * 128:(m % 4 + 1) * 128],
                        ident[:],
                    )
                    vcopy(v_aug[m][:, 0:DV], vps[:])

            def scores(kgs):
                # S^T for key-slot k over q-group g (positions >= 2k)
                for k, g in kgs:
                    a = max(2 * k - 4 * g, 0)
                    if a > 3:
                        continue
                    st = mm_ps.tile([128, 512], F32, tag="mm")
                    nc.tensor.matmul(
                        st[:, a * 128:512],
                        kt[k // 4][:, (k % 4) * 128:(k % 4 + 1) * 128],
                        qt[g][:, a * 128:512],
                        start=True, stop=True,
                        skip_group_check=True,
                    )
                    nc.scalar.activation(
                        at[(k, g)][:, a * 128:512],
                        st[:, a * 128:512],
                        mybir.ActivationFunctionType.Exp,
                        bias=0.0, scale=1.0,
                    )
                    if g == k // 2:
                        # causal boundary (post-exp 0/1 multiplies):
                        # position 2k = diagonal triangle (both parities),
                        # 2k+1 = live/dead by core parity (mask data)
                        for blk, moff in ((2 * k, 0), (2 * k + 1, 128)):
                            p = (blk % 4) * 128
                            asl = at[(k, g)][:, p:p + 128]
                            nc.gpsimd.tensor_mul(
                                asl, asl, mask_sb[:, moff:moff + 128]
                            )

            def av(poss):
                for pos in poss:
                    g, q = pos // 4, (pos % 4) * 128
                    zp = z_ps.tile([128, DV + 1], F32, tag="z")
                    kmax = pos // 2
                    for k in range(kmax + 1):
                        nc.tensor.matmul(
                            zp[:],
                            at[(k, g)][:, q:q + 128],
                            v_aug[k][:],
                            start=(k == 0), stop=(k == kmax),
                        )
                    stg = work.tile([128, DV + 1], F32, tag="zout")
                    vcopy(stg[:], zp[:])
                    nc.sync.dma_start(
                        out=out_ext[pos * 128:(pos + 1) * 128, :],
                        in_=stg[:],
                    )

            # ---- rounds: compute rides just behind the input stream ----
            for pr in range(4):
                qproj(pr)
                kvproj(pr)
                vt_blocks([2 * pr, 2 * pr + 1])
                if pr == 0:
                    warm(3)
                scores([(k, pr) for k in range(2 * pr + 2)])
                av(range(4 * pr, 4 * pr + 4))

    nc.finalize()
    return nc


_NC = None


def _get_nc():
    global _NC
    if _NC is None:
        _NC = build_nc()
    return _NC


def _make_masks():
    npdt = _np_cdt()
    p = np.arange(128)[:, None]   # key (partition)
    q = np.arange(128)[None, :]   # query (free)
    triT = (p <= q).astype(npdt)  # diagonal block, transposed
    dead = np.zeros((128, 128), npdt)
    live = np.ones((128, 128), npdt)
    mask_even = np.concatenate([triT, live], axis=1)
    mask_odd = np.concatenate([triT, dead], axis=1)
    return mask_even, mask_odd


def kernel(X, W_Q, W_K, W_V):
    X = np.asarray(X, np.float32)
    W_Q = np.asarray(W_Q, np.float32)
    W_K = np.asarray(W_K, np.float32)
    W_V = np.asarray(W_V, np.float32)

    nc = _get_nc()
    npdt = _np_cdt()
    mask_even, mask_odd = _make_masks()

    def warr(W):
        return np.ascontiguousarray(
            W.astype(npdt).reshape(NCH, 128, DK).transpose(1, 0, 2)
            .reshape(128, NCH * DK)
        )

    wq = warr(W_Q * SCALE)   # fold the 1/sqrt(d_k) into W_Q
    wk = warr(W_K)
    wv = warr(W_V)

    in_maps = []
    for c in range(8):
        b, kp = c // 2, c % 2
        # [c, p, j, l]: d_model chunks x partitions x q-blocks x 128
        xt = (np.ascontiguousarray(X[b].T).astype(npdt)
              .reshape(NCH, 128, NB, 128))
        if kp:
            # odd cores: swap each q-block pair so the KEY block is first
            xt = xt[:, :, [j + 1 - 2 * (j % 2) for j in range(NB)], :]
        m = {"wq": wq, "wk": wk, "wv": wv,
             "maskT": mask_odd if kp else mask_even}
        for i in range(8):
            # pack piece i as [p, c*256+l]
            m[f"xp{i}"] = np.ascontiguousarray(
                xt[:, :, 2 * i:2 * i + 2, :]
                .transpose(1, 0, 2, 3).reshape(128, NCH * 256)
            )
        in_maps.append(m)

    res = run_bass_kernel_spmd(nc, in_maps, list(range(8)))

    Z = np.zeros((B, L, DV), np.float32)
    for b in range(B):
        oe = res.results[2 * b]["out"].reshape(NB, 128, DV + 1)
        oo = res.results[2 * b + 1]["out"].reshape(NB, 128, DV + 1)
        # un-permute: even core position p holds q-block p; odd core
        # position 2t holds q-block 2t+1, position 2t+1 holds 2t
        perm = [j + 1 - 2 * (j % 2) for j in range(NB)]
        oo = oo[perm]
        tot = oe + oo
        Z[b] = (tot[:, :, 0:DV] / tot[:, :, DV:DV + 1]).reshape(L, DV)
    return Z
